# revision 40
# baseline (speedup 1.0000x reference)
"""AdaptiveAggGCN Trainium2 kernel (8 NeuronCores, data/graph-parallel).

Math: for each of G=3 graphs,
    y_i = D_in^{-1/2} A_i D_out^{-1/2} x          (sparse normalized aggregation)
    h_i = tanh(a_i * (y_i @ W_i + b_i)),  a = softmax(alphas)
    out = concat(h_i) @ W_lin + b_lin

Since row-scaling and the segment-sum commute with the dense matmuls, the
aggregation is done in the 512-wide input domain first.  Nodes are sharded
across the 8 cores by destination row (3750 each); every core gets the full
(bf16-cast) x in DRAM, gathers the source rows its edges need with
`dma_gather`, and reduces them per 128-destination block with a TensorEngine
matmul against a host-built sparse selector S (one column per edge slot,
value = a_i * rsqrt(deg_in[dst]) * rsqrt(deg_out[src])).  The dense W / W_lin
matmuls, tanh and the final output transpose all run on-chip in bf16 with
fp32 PSUM accumulation.  No collectives are needed: each core owns its output
rows and the host concatenates the 8 shards.
"""

import numpy as np
import ml_dtypes

import concourse.bacc as bacc
import concourse.bass as bass
import concourse.mybir as mybir
import concourse.tile as tile
from concourse.bass_utils import run_bass_kernel_spmd
from concourse.library_config import mlp as _mlp_lib

BF16 = ml_dtypes.bfloat16

N, IN, HID, OUT, G = 30000, 512, 512, 256, 3
NCORES = 8
SHARD = N // NCORES          # 3750 nodes per core
PAD = 4096                   # padded node count per core (8 tiles of 512)
NT = PAD // 512              # 8 node tiles for the dense matmuls
NBLK = 30                    # 128-dst blocks that can hold edges (30*128=3840)
BS = 128

_CACHE: dict = {}


def _softmax(v):
    v = np.asarray(v, np.float64)
    e = np.exp(v - v.max())
    return (e / e.sum()).astype(np.float32)


def _balance_blocks(localdeg):
    """Assign SHARD dst rows to NBLK blocks of <=128 rows, greedily equalizing
    the per-graph edge load across blocks (multi-graph LPT).  Returns
    devrow[d] = on-device row (block*128 + row) for local dst d."""
    order = np.argsort(-localdeg.sum(axis=1), kind="stable")
    loads = np.zeros((NBLK, G), np.int64)
    rows = np.zeros(NBLK, np.int64)
    blk = np.zeros(SHARD, np.int64)
    row = np.zeros(SHARD, np.int64)
    for d in order:
        cand = (loads + localdeg[d]).max(axis=1).astype(np.float64)
        cand[rows >= BS] = np.inf
        j = int(np.argmin(cand))
        blk[d] = j
        row[d] = rows[j]
        rows[j] += 1
        loads[j] += localdeg[d]
    return blk, row, blk * BS + row


def _prepare(inputs):
    """Host-side sharding/layout: returns (in_maps, K, devrows) where K[i][b]
    is the number of 128-edge chunks for graph i, dst-block b (same on all
    cores) and devrows[c][d] maps local dst d to its on-device row."""
    x = np.ascontiguousarray(np.asarray(inputs["x"], np.float32))
    xb = x.astype(BF16)
    a = _softmax(inputs["alphas"])

    ident = np.eye(128, dtype=BF16)
    # bias layout: cols 0..11 -> a_i*b_i per 128-chunk, cols 12..13 -> b_lin
    bias = np.zeros((128, 14), np.float32)
    for i in range(G):
        bi = np.asarray(inputs[f"b{i}"], np.float32) * a[i]
        bias[:, i * 4:(i + 1) * 4] = bi.reshape(4, 128).T
    bias[:, 12:14] = np.asarray(inputs["b_lin"], np.float32).reshape(2, 128).T

    # Per-core balanced dst-row -> block assignment shared by all 3 graphs.
    edges = []
    for i in range(G):
        e = np.asarray(inputs[f"edges{i}"]).astype(np.int64)
        edges.append(e)
    blk_of = []
    row_of = []
    devrows = []
    for c in range(NCORES):
        localdeg = np.zeros((SHARD, G), np.int64)
        for i in range(G):
            dst = edges[i][1]
            m = (dst >= c * SHARD) & (dst < (c + 1) * SHARD)
            localdeg[:, i] = np.bincount(dst[m] - c * SHARD, minlength=SHARD)
        blk, row, devrow = _balance_blocks(localdeg)
        blk_of.append(blk)
        row_of.append(row)
        devrows.append(devrow)

    K_all = []
    idx_arrs = [[None] * G for _ in range(NCORES)]
    s_arrs = [[None] * G for _ in range(NCORES)]
    for i in range(G):
        src, dst = edges[i][0], edges[i][1]
        deg_out = np.maximum(np.bincount(src, minlength=N), 1).astype(np.float64)
        deg_in = np.maximum(np.bincount(dst, minlength=N), 1).astype(np.float64)
        w = (a[i] / np.sqrt(deg_in[dst] * deg_out[src])).astype(np.float32)

        core = dst // SHARD
        d_loc = dst - core * SHARD
        ec = np.arange(len(dst))
        blk = np.empty(len(dst), np.int64)
        drow = np.empty(len(dst), np.int64)
        for c in range(NCORES):
            m = core == c
            blk[m] = blk_of[c][d_loc[m]]
            drow[m] = row_of[c][d_loc[m]]
        key = core * NBLK + blk
        order = np.argsort(key, kind="stable")
        src_s, w_s, drow_s = src[order], w[order], drow[order]
        cnts = np.bincount(key, minlength=NCORES * NBLK).reshape(NCORES, NBLK)
        K = np.maximum(np.ceil(cnts.max(axis=0) / BS), 1).astype(np.int64)  # [NBLK]
        K_all.append(K.tolist())
        CH = np.concatenate([[0], np.cumsum(K)])  # chunk offsets, CH[NBLK] = TK
        TK = int(CH[NBLK])
        CW = 8 * TK          # idx columns (int16, 16-wrapped)
        CS = 128 * TK        # S columns per partition

        starts = np.concatenate([[0], np.cumsum(cnts.reshape(-1))])
        for c in range(NCORES):
            idx16 = np.zeros((128, CW), np.int16)
            s_mat = np.zeros((128, CS), BF16)
            for b in range(NBLK):
                gidx = c * NBLK + b
                s0, s1 = starts[gidx], starts[gidx + 1]
                cnt = s1 - s0
                if cnt == 0:
                    continue
                sb = src_s[s0:s1]
                db = drow_s[s0:s1]
                wb = w_s[s0:s1]
                # ascending source addresses within the gather improve HBM
                # locality; slot order is free (S permutes with it)
                so = np.argsort(sb, kind="stable")
                sb, db, wb = sb[so], db[so], wb[so]
                slots = 128 * int(K[b])
                # padding slots gather row 0 with weight 0 (all slots valid,
                # so the matmul never reads unwritten SBUF)
                fl = np.zeros(slots, np.int16)
                fl[:cnt] = sb.astype(np.int16)
                # [16, 8K] block, replicated across all 128 partitions — queue q's
                # Q7 descriptor-gen CPU pair reads partitions [32q, 32q+32)
                idx16[:, 8 * CH[b]: 8 * CH[b + 1]] = np.tile(fl.reshape(-1, 16).T, (8, 1))
                j = np.arange(cnt)
                s_mat[j % 128, 128 * CH[b] + (j // 128) * 128 + db] = wb.astype(BF16)
            idx_arrs[c][i] = idx16
            s_arrs[c][i] = s_mat

    in_maps = []
    for c in range(NCORES):
        m = {
            "xb": xb,
            "ident": ident,
            "bias": bias,
            "wl": np.ascontiguousarray(np.asarray(inputs["W_lin"], np.float32)).astype(BF16),
        }
        for i in range(G):
            m[f"w{i}"] = np.ascontiguousarray(np.asarray(inputs[f"W{i}"], np.float32)).astype(BF16)
            m[f"idx{i}"] = idx_arrs[c][i]
            m[f"s{i}"] = s_arrs[c][i]
        in_maps.append(m)
    return in_maps, K_all, devrows


def _build(K_all):
    """Build the single-core (SPMD) Bass graph for chunk layout K_all."""
    f32 = mybir.dt.float32
    bf16 = mybir.dt.bfloat16
    i16 = mybir.dt.int16
    TANH = mybir.ActivationFunctionType.Tanh
    IDENT = mybir.ActivationFunctionType.Identity

    CH_all = [np.concatenate([[0], np.cumsum(K)]).astype(int) for K in K_all]
    TK = [int(CH_all[i][NBLK]) for i in range(G)]

    nc = bacc.Bacc("TRN2", target_bir_lowering=False, debug=False, num_swdge_queues=4)
    xb_d = nc.dram_tensor("xb", [N, IN], bf16, kind="ExternalInput")
    ident_d = nc.dram_tensor("ident", [128, 128], bf16, kind="ExternalInput")
    bias_d = nc.dram_tensor("bias", [128, 14], f32, kind="ExternalInput")
    wl_d = nc.dram_tensor("wl", [G * HID, OUT], bf16, kind="ExternalInput")
    w_d = [nc.dram_tensor(f"w{i}", [IN, HID], bf16, kind="ExternalInput") for i in range(G)]
    idx_d = [nc.dram_tensor(f"idx{i}", [128, 8 * TK[i]], i16, kind="ExternalInput") for i in range(G)]
    s_d = [nc.dram_tensor(f"s{i}", [128, 128 * TK[i]], bf16, kind="ExternalInput") for i in range(G)]
    out_d = nc.dram_tensor("out", [PAD, OUT], f32, kind="ExternalOutput")

    with tile.TileContext(nc) as tc:
        with (
            tc.tile_pool(name="const", bufs=1) as cp,
            tc.tile_pool(name="work", bufs=1) as wp,
            tc.tile_pool(name="ps", bufs=1, space="PSUM") as pp,
        ):
            # idx tiles first: the first gather only needs the first chunk of
            # idx0, so those DMAs go ahead of the (larger) constant loads
            idxts = []
            for i in range(G):
                idxt = wp.tile([128, 8 * TK[i]], i16, name=f"idxt{i}", tag=f"idx{i}", bufs=1)
                nchunk = 8 if i == 0 else 4
                cw = 8 * TK[i]
                step = -(-cw // nchunk)
                for c0 in range(0, cw, step):
                    nc.sync.dma_start(idxt[:, c0:min(c0 + step, cw)],
                                      idx_d[i].ap()[:, c0:min(c0 + step, cw)])
                idxts.append(idxt)
            ident = cp.tile([128, 128], bf16, name="identsb")
            nc.sync.dma_start(ident[:], ident_d.ap())
            bias = cp.tile([128, 14], f32, name="biassb")
            nc.sync.dma_start(bias[:], bias_d.ap())
            wl = cp.tile([128, 12, OUT], bf16, name="wlsb")
            nc.sync.dma_start(wl[:], wl_d.ap().rearrange("(c p) o -> p c o", p=128))
            wsb = []
            for i in range(G):
                wi = cp.tile([128, 4, HID], bf16, name=f"wsb{i}")
                nc.sync.dma_start(wi[:], w_d[i].ap().rearrange("(c p) o -> p c o", p=128))
                wsb.append(wi)
            # fp32 accumulator for the output projection, summed over graphs
            oacc = cp.tile([128, 2, PAD], f32, name="oacc")

            nc.gpsimd.load_library(_mlp_lib)

            qn = 0
            BPG = 2  # dst blocks per dma_gather (amortizes ~1us fixed overhead)
            # gather/S tile depths scale with the worst pair size so a skewed
            # edge distribution cannot blow the SBUF budget
            kt_max = max(
                sum(int(K_all[i][b]) for b in range(bp, min(bp + BPG, NBLK)))
                for i in range(G) for bp in range(0, NBLK, BPG)
            )
            g_bufs = max(3, min(8, (60 * 1024) // (kt_max * IN * 2)))
            s_bufs = max(2, min(4, (12 * 1024) // (kt_max * 128 * 2)))
            for i in range(G):
                CH = CH_all[i]
                idxt = idxts[i]

                # One node tile (512 nodes = 4 dst blocks) at a time: aggregate,
                # transpose, then immediately run the dense W / output-projection
                # matmuls for that tile so PE has dense work while the next
                # gathers are in flight.
                for nt in range(NT):
                    ytn = wp.tile([128, 4, 512], bf16, name=f"ytn{i}_{nt}", tag="ytn", bufs=4)
                    b_lo = nt * 4
                    b_hi = min(b_lo + 4, NBLK)
                    if b_hi - b_lo < 4:
                        # node range past the last real dst block: zero-fill
                        nc.vector.memset(ytn[:, :, (b_hi - b_lo) * BS:], 0.0)
                    for bp in range(b_lo, b_hi, BPG):
                        blks = [b for b in range(bp, min(bp + BPG, b_hi))]
                        Ks = [int(K_all[i][b]) for b in blks]
                        Ktot = sum(Ks)
                        gt = wp.tile([128, Ktot, IN], bf16, name=f"g{i}_{bp}", tag="g", bufs=g_bufs)
                        st = wp.tile([128, Ktot, 128], bf16, name=f"s{i}_{bp}", tag="s", bufs=s_bufs)
                        nc.sync.dma_start(
                            st[:],
                            s_d[i].ap()[:, 128 * CH[blks[0]]: 128 * CH[blks[-1] + 1]]
                            .rearrange("p (k d) -> p k d", d=128),
                        )
                        nc.gpsimd.dma_gather(
                            gt[:], xb_d.ap(), idxt[:, 8 * CH[blks[0]]: 8 * CH[blks[-1] + 1]],
                            128 * Ktot, 128 * Ktot, IN, queue_num=qn,
                            single_packet=False,
                        )
                        qn = (qn + 1) % 4
                        koff = 0
                        for bi, b in enumerate(blks):
                            Kb = Ks[bi]
                            py = pp.tile([128, 512], f32, name=f"py{i}_{b}", tag="yacc", bufs=2)
                            for k in range(Kb):
                                nc.tensor.matmul(py[:], st[:, koff + k, :], gt[:, koff + k, :],
                                                 start=(k == 0), stop=(k == Kb - 1))
                            koff += Kb
                            ysb = wp.tile([128, 512], bf16, name=f"y{i}_{b}", tag="ysb", bufs=2)
                            nc.scalar.copy(ysb[:], py[:])
                            pyt = pp.tile([128, 4, 128], bf16, name=f"pyt{i}_{b}", tag="ytp", bufs=2)
                            for fc in range(4):
                                nc.tensor.matmul(pyt[:, fc, :], ysb[:, fc * 128:(fc + 1) * 128],
                                                 ident[:], is_transpose=True,
                                                 start=(fc == 0), stop=(fc == 3))
                            nc.vector.tensor_copy(ytn[:, :, (b - b_lo) * BS:(b - b_lo + 1) * BS],
                                                  pyt[:])

                    htile = wp.tile([128, 4, 512], bf16, name=f"ht{i}_{nt}", tag="ht", bufs=4)
                    for of in range(4):
                        ph = pp.tile([128, 512], f32, name=f"ph{i}_{nt}_{of}", tag="acc", bufs=2)
                        for c4 in range(4):
                            nc.tensor.matmul(ph[:], wsb[i][:, c4, of * 128:(of + 1) * 128],
                                             ytn[:, c4, :],
                                             start=(c4 == 0), stop=(c4 == 3))
                        nc.scalar.activation(htile[:, of, :], ph[:],
                                             TANH, bias=bias[:, i * 4 + of: i * 4 + of + 1],
                                             scale=1.0)
                    for om in range(2):
                        pacc = pp.tile([128, 512], f32, name=f"pa{i}_{nt}_{om}", tag="acc", bufs=2)
                        for hf in range(4):
                            nc.tensor.matmul(pacc[:], wl[:, i * 4 + hf, om * 128:(om + 1) * 128],
                                             htile[:, hf, :],
                                             start=(hf == 0), stop=(hf == 3))
                        dst = oacc[:, om, nt * 512:(nt + 1) * 512]
                        if i == 0:
                            nc.vector.tensor_copy(dst, pacc[:])
                        else:
                            nc.vector.tensor_add(dst, dst, pacc[:])

                    if i == G - 1:
                        # output epilogue for this node tile: +b_lin, transpose
                        # back to node-major, DMA out
                        po = pp.tile([128, 4, OUT], bf16, name=f"po{nt}", tag="otp", bufs=2)
                        otb = []
                        for om in range(2):
                            ob = wp.tile([128, 512], bf16, name=f"ot{nt}_{om}", tag="ot", bufs=2)
                            nc.scalar.activation(ob[:], oacc[:, om, nt * 512:(nt + 1) * 512],
                                                 IDENT, bias=bias[:, 12 + om: 13 + om],
                                                 scale=1.0)
                            otb.append(ob)
                        n8 = 0
                        for om in range(2):
                            for j in range(4):
                                nc.tensor.matmul(po[:, j, om * 128:(om + 1) * 128],
                                                 otb[om][:, j * 128:(j + 1) * 128], ident[:],
                                                 is_transpose=True,
                                                 start=(n8 == 0), stop=(n8 == 7))
                                n8 += 1
                        off = wp.tile([128, 4, OUT], f32, name=f"off{nt}", tag="of", bufs=2)
                        nc.vector.tensor_copy(off[:], po[:])
                        nc.sync.dma_start(
                            out_d.ap().rearrange("(t j p) o -> t p j o", p=128, j=4)[nt],
                            off[:],
                        )

    nc.compile()
    return nc


def kernel(**inputs) -> np.ndarray:
    in_maps, K_all, devrows = _prepare(inputs)
    key = tuple(tuple(k) for k in K_all)
    nc = _CACHE.get(key)
    if nc is None:
        nc = _build(K_all)
        _CACHE.clear()
        _CACHE[key] = nc
    res = run_bass_kernel_spmd(nc, in_maps, core_ids=list(range(NCORES)))
    out = np.concatenate(
        [np.asarray(res.results[c]["out"])[devrows[c]] for c in range(NCORES)], axis=0
    )
    return out.astype(np.float32)


# revision 41
# speedup vs baseline: 1.0619x; 1.0619x over previous
"""AdaptiveAggGCN Trainium2 kernel (8 NeuronCores, data/graph-parallel).

Math: for each of G=3 graphs,
    y_i = D_in^{-1/2} A_i D_out^{-1/2} x          (sparse normalized aggregation)
    h_i = tanh(a_i * (y_i @ W_i + b_i)),  a = softmax(alphas)
    out = concat(h_i) @ W_lin + b_lin

Since row-scaling and the segment-sum commute with the dense matmuls, the
aggregation is done in the 512-wide input domain first.  Nodes are sharded
across the 8 cores by destination row (3750 each); every core gets the full
(bf16-cast) x in DRAM, gathers the source rows its edges need with
`dma_gather`, and reduces them per 128-destination block with a TensorEngine
matmul against a host-built sparse selector S (one column per edge slot,
value = a_i * rsqrt(deg_in[dst]) * rsqrt(deg_out[src])).  The dense W / W_lin
matmuls, tanh and the final output transpose all run on-chip in bf16 with
fp32 PSUM accumulation.  No collectives are needed: each core owns its output
rows and the host concatenates the 8 shards.
"""

import numpy as np
import ml_dtypes

import concourse.bacc as bacc
import concourse.bass as bass
import concourse.mybir as mybir
import concourse.tile as tile
from concourse.bass_utils import run_bass_kernel_spmd
from concourse.library_config import mlp as _mlp_lib

BF16 = ml_dtypes.bfloat16

N, IN, HID, OUT, G = 30000, 512, 512, 256, 3
NCORES = 8
SHARD = N // NCORES          # 3750 nodes per core
PAD = 4096                   # padded node count per core (8 tiles of 512)
NT = PAD // 512              # 8 node tiles for the dense matmuls
NBLK = 30                    # 128-dst blocks that can hold edges (30*128=3840)
BS = 128

_CACHE: dict = {}


def _softmax(v):
    v = np.asarray(v, np.float64)
    e = np.exp(v - v.max())
    return (e / e.sum()).astype(np.float32)


def _balance_blocks(localdeg):
    """Assign SHARD dst rows to NBLK blocks of <=128 rows, greedily equalizing
    the per-graph edge load across blocks (multi-graph LPT).  Returns
    devrow[d] = on-device row (block*128 + row) for local dst d."""
    order = np.argsort(-localdeg.sum(axis=1), kind="stable")
    loads = np.zeros((NBLK, G), np.int64)
    rows = np.zeros(NBLK, np.int64)
    blk = np.zeros(SHARD, np.int64)
    row = np.zeros(SHARD, np.int64)
    for d in order:
        cand = (loads + localdeg[d]).max(axis=1).astype(np.float64)
        cand[rows >= BS] = np.inf
        j = int(np.argmin(cand))
        blk[d] = j
        row[d] = rows[j]
        rows[j] += 1
        loads[j] += localdeg[d]
    return blk, row, blk * BS + row


def _prepare(inputs):
    """Host-side sharding/layout: returns (in_maps, K, devrows) where K[i][b]
    is the number of 128-edge chunks for graph i, dst-block b (same on all
    cores) and devrows[c][d] maps local dst d to its on-device row."""
    x = np.ascontiguousarray(np.asarray(inputs["x"], np.float32))
    xb = x.astype(BF16)
    a = _softmax(inputs["alphas"])

    ident = np.eye(128, dtype=BF16)
    # bias layout: cols 0..11 -> a_i*b_i per 128-chunk, cols 12..13 -> b_lin
    bias = np.zeros((128, 14), np.float32)
    for i in range(G):
        bi = np.asarray(inputs[f"b{i}"], np.float32) * a[i]
        bias[:, i * 4:(i + 1) * 4] = bi.reshape(4, 128).T
    bias[:, 12:14] = np.asarray(inputs["b_lin"], np.float32).reshape(2, 128).T

    # Per-core balanced dst-row -> block assignment shared by all 3 graphs.
    edges = []
    for i in range(G):
        e = np.asarray(inputs[f"edges{i}"]).astype(np.int64)
        edges.append(e)
    blk_of = []
    row_of = []
    devrows = []
    for c in range(NCORES):
        localdeg = np.zeros((SHARD, G), np.int64)
        for i in range(G):
            dst = edges[i][1]
            m = (dst >= c * SHARD) & (dst < (c + 1) * SHARD)
            localdeg[:, i] = np.bincount(dst[m] - c * SHARD, minlength=SHARD)
        blk, row, devrow = _balance_blocks(localdeg)
        blk_of.append(blk)
        row_of.append(row)
        devrows.append(devrow)

    K_all = []
    idx_arrs = [[None] * G for _ in range(NCORES)]
    s_arrs = [[None] * G for _ in range(NCORES)]
    for i in range(G):
        src, dst = edges[i][0], edges[i][1]
        deg_out = np.maximum(np.bincount(src, minlength=N), 1).astype(np.float64)
        deg_in = np.maximum(np.bincount(dst, minlength=N), 1).astype(np.float64)
        w = (a[i] / np.sqrt(deg_in[dst] * deg_out[src])).astype(np.float32)

        core = dst // SHARD
        d_loc = dst - core * SHARD
        ec = np.arange(len(dst))
        blk = np.empty(len(dst), np.int64)
        drow = np.empty(len(dst), np.int64)
        for c in range(NCORES):
            m = core == c
            blk[m] = blk_of[c][d_loc[m]]
            drow[m] = row_of[c][d_loc[m]]
        key = core * NBLK + blk
        order = np.argsort(key, kind="stable")
        src_s, w_s, drow_s = src[order], w[order], drow[order]
        cnts = np.bincount(key, minlength=NCORES * NBLK).reshape(NCORES, NBLK)
        K = np.maximum(np.ceil(cnts.max(axis=0) / BS), 1).astype(np.int64)  # [NBLK]
        K_all.append(K.tolist())
        CH = np.concatenate([[0], np.cumsum(K)])  # chunk offsets, CH[NBLK] = TK
        TK = int(CH[NBLK])
        CW = 8 * TK          # idx columns (int16, 16-wrapped)
        CS = 128 * TK        # S columns per partition

        starts = np.concatenate([[0], np.cumsum(cnts.reshape(-1))])
        for c in range(NCORES):
            idx16 = np.zeros((128, CW), np.int16)
            s_mat = np.zeros((128, CS), BF16)
            for b in range(NBLK):
                gidx = c * NBLK + b
                s0, s1 = starts[gidx], starts[gidx + 1]
                cnt = s1 - s0
                if cnt == 0:
                    continue
                sb = src_s[s0:s1]
                db = drow_s[s0:s1]
                wb = w_s[s0:s1]
                # ascending source addresses within the gather improve HBM
                # locality; slot order is free (S permutes with it)
                so = np.argsort(sb, kind="stable")
                sb, db, wb = sb[so], db[so], wb[so]
                slots = 128 * int(K[b])
                # padding slots gather row 0 with weight 0 (all slots valid,
                # so the matmul never reads unwritten SBUF)
                fl = np.zeros(slots, np.int16)
                fl[:cnt] = sb.astype(np.int16)
                # [16, 8K] block, replicated across all 128 partitions — queue q's
                # Q7 descriptor-gen CPU pair reads partitions [32q, 32q+32)
                idx16[:, 8 * CH[b]: 8 * CH[b + 1]] = np.tile(fl.reshape(-1, 16).T, (8, 1))
                j = np.arange(cnt)
                s_mat[j % 128, 128 * CH[b] + (j // 128) * 128 + db] = wb.astype(BF16)
            idx_arrs[c][i] = idx16
            s_arrs[c][i] = s_mat

    in_maps = []
    for c in range(NCORES):
        m = {
            "xb": xb,
            "ident": ident,
            "bias": bias,
            "wl": np.ascontiguousarray(np.asarray(inputs["W_lin"], np.float32)).astype(BF16),
        }
        for i in range(G):
            m[f"w{i}"] = np.ascontiguousarray(np.asarray(inputs[f"W{i}"], np.float32)).astype(BF16)
            m[f"idx{i}"] = idx_arrs[c][i]
            m[f"s{i}"] = s_arrs[c][i]
        in_maps.append(m)
    return in_maps, K_all, devrows


def _build(K_all):
    """Build the single-core (SPMD) Bass graph for chunk layout K_all."""
    f32 = mybir.dt.float32
    bf16 = mybir.dt.bfloat16
    i16 = mybir.dt.int16
    TANH = mybir.ActivationFunctionType.Tanh
    IDENT = mybir.ActivationFunctionType.Identity

    CH_all = [np.concatenate([[0], np.cumsum(K)]).astype(int) for K in K_all]
    TK = [int(CH_all[i][NBLK]) for i in range(G)]

    nc = bacc.Bacc("TRN2", target_bir_lowering=False, debug=False, num_swdge_queues=4)
    xb_d = nc.dram_tensor("xb", [N, IN], bf16, kind="ExternalInput")
    ident_d = nc.dram_tensor("ident", [128, 128], bf16, kind="ExternalInput")
    bias_d = nc.dram_tensor("bias", [128, 14], f32, kind="ExternalInput")
    wl_d = nc.dram_tensor("wl", [G * HID, OUT], bf16, kind="ExternalInput")
    w_d = [nc.dram_tensor(f"w{i}", [IN, HID], bf16, kind="ExternalInput") for i in range(G)]
    idx_d = [nc.dram_tensor(f"idx{i}", [128, 8 * TK[i]], i16, kind="ExternalInput") for i in range(G)]
    s_d = [nc.dram_tensor(f"s{i}", [128, 128 * TK[i]], bf16, kind="ExternalInput") for i in range(G)]
    out_d = nc.dram_tensor("out", [PAD, OUT], f32, kind="ExternalOutput")

    with tile.TileContext(nc) as tc:
        with (
            tc.tile_pool(name="const", bufs=1) as cp,
            tc.tile_pool(name="work", bufs=1) as wp,
            tc.tile_pool(name="ps", bufs=1, space="PSUM") as pp,
        ):
            # idx tiles first: the first gather only needs the first chunk of
            # idx0, so those DMAs go ahead of the (larger) constant loads
            idxts = []
            for i in range(G):
                idxt = wp.tile([128, 8 * TK[i]], i16, name=f"idxt{i}", tag=f"idx{i}", bufs=1)
                nchunk = 8 if i == 0 else 4
                cw = 8 * TK[i]
                step = -(-cw // nchunk)
                for c0 in range(0, cw, step):
                    nc.sync.dma_start(idxt[:, c0:min(c0 + step, cw)],
                                      idx_d[i].ap()[:, c0:min(c0 + step, cw)])
                idxts.append(idxt)
            ident = cp.tile([128, 128], bf16, name="identsb")
            nc.sync.dma_start(ident[:], ident_d.ap())
            bias = cp.tile([128, 14], f32, name="biassb")
            nc.sync.dma_start(bias[:], bias_d.ap())
            wl = cp.tile([128, 12, OUT], bf16, name="wlsb")
            nc.sync.dma_start(wl[:], wl_d.ap().rearrange("(c p) o -> p c o", p=128))
            wsb = []
            for i in range(G):
                wi = cp.tile([128, 4, HID], bf16, name=f"wsb{i}")
                nc.sync.dma_start(wi[:], w_d[i].ap().rearrange("(c p) o -> p c o", p=128))
                wsb.append(wi)
            # fp32 accumulator for the output projection, summed over graphs
            oacc = cp.tile([128, 2, PAD], f32, name="oacc")

            nc.gpsimd.load_library(_mlp_lib)

            qn = 0
            BPG = 2  # dst blocks per dma_gather (amortizes ~1us fixed overhead)
            # gather/S tile depths scale with the worst pair size so a skewed
            # edge distribution cannot blow the SBUF budget
            kt_max = max(
                sum(int(K_all[i][b]) for b in range(bp, min(bp + BPG, NBLK)))
                for i in range(G) for bp in range(0, NBLK, BPG)
            )
            g_bufs = max(3, min(8, (60 * 1024) // (kt_max * IN * 2)))
            s_bufs = max(2, min(4, (12 * 1024) // (kt_max * 128 * 2)))
            for i in range(G):
                CH = CH_all[i]
                idxt = idxts[i]

                # One node tile (512 nodes = 4 dst blocks) at a time: aggregate,
                # transpose, then immediately run the dense W / output-projection
                # matmuls for that tile so PE has dense work while the next
                # gathers are in flight.
                for nt in range(NT):
                    ytn = wp.tile([128, 4, 512], bf16, name=f"ytn{i}_{nt}", tag="ytn", bufs=4)
                    b_lo = nt * 4
                    b_hi = min(b_lo + 4, NBLK)
                    if b_hi - b_lo < 4:
                        # node range past the last real dst block: zero-fill
                        nc.vector.memset(ytn[:, :, (b_hi - b_lo) * BS:], 0.0)
                    for bp in range(b_lo, b_hi, BPG):
                        blks = [b for b in range(bp, min(bp + BPG, b_hi))]
                        Ks = [int(K_all[i][b]) for b in blks]
                        Ktot = sum(Ks)
                        gt = wp.tile([128, Ktot, IN], bf16, name=f"g{i}_{bp}", tag="g", bufs=g_bufs)
                        st = wp.tile([128, Ktot, 128], bf16, name=f"s{i}_{bp}", tag="s", bufs=s_bufs)
                        nc.sync.dma_start(
                            st[:],
                            s_d[i].ap()[:, 128 * CH[blks[0]]: 128 * CH[blks[-1] + 1]]
                            .rearrange("p (k d) -> p k d", d=128),
                        )
                        nc.gpsimd.dma_gather(
                            gt[:], xb_d.ap(), idxt[:, 8 * CH[blks[0]]: 8 * CH[blks[-1] + 1]],
                            128 * Ktot, 128 * Ktot, IN, queue_num=qn,
                            single_packet=False,
                        )
                        qn = (qn + 1) % 4
                        koff = 0
                        for bi, b in enumerate(blks):
                            Kb = Ks[bi]
                            py = pp.tile([128, 512], f32, name=f"py{i}_{b}", tag="acc", bufs=3)
                            for k in range(Kb):
                                nc.tensor.matmul(py[:], st[:, koff + k, :], gt[:, koff + k, :],
                                                 start=(k == 0), stop=(k == Kb - 1))
                            koff += Kb
                            ysb = wp.tile([128, 512], bf16, name=f"y{i}_{b}", tag="ysb", bufs=2)
                            nc.vector.tensor_copy(ysb[:], py[:])
                            pyt = pp.tile([128, 4, 128], bf16, name=f"pyt{i}_{b}", tag="ytp", bufs=2)
                            for fc in range(4):
                                nc.tensor.matmul(pyt[:, fc, :], ysb[:, fc * 128:(fc + 1) * 128],
                                                 ident[:], is_transpose=True,
                                                 start=(fc == 0), stop=(fc == 3))
                            nc.vector.tensor_copy(ytn[:, :, (b - b_lo) * BS:(b - b_lo + 1) * BS],
                                                  pyt[:])

                    htile = wp.tile([128, 4, 512], bf16, name=f"ht{i}_{nt}", tag="ht", bufs=4)
                    for of in range(4):
                        ph = pp.tile([128, 512], f32, name=f"ph{i}_{nt}_{of}", tag="acc", bufs=3)
                        for c4 in range(4):
                            nc.tensor.matmul(ph[:], wsb[i][:, c4, of * 128:(of + 1) * 128],
                                             ytn[:, c4, :],
                                             start=(c4 == 0), stop=(c4 == 3))
                        nc.scalar.activation(htile[:, of, :], ph[:],
                                             TANH, bias=bias[:, i * 4 + of: i * 4 + of + 1],
                                             scale=1.0)
                    for om in range(2):
                        pacc = pp.tile([128, 512], f32, name=f"pa{i}_{nt}_{om}", tag="acc", bufs=3)
                        for hf in range(4):
                            nc.tensor.matmul(pacc[:], wl[:, i * 4 + hf, om * 128:(om + 1) * 128],
                                             htile[:, hf, :],
                                             start=(hf == 0), stop=(hf == 3))
                        dst = oacc[:, om, nt * 512:(nt + 1) * 512]
                        if i == 0:
                            nc.vector.tensor_copy(dst, pacc[:])
                        else:
                            nc.vector.tensor_add(dst, dst, pacc[:])

                    if i == G - 1:
                        # output epilogue for this node tile: +b_lin, transpose
                        # back to node-major, DMA out
                        po = pp.tile([128, 4, OUT], bf16, name=f"po{nt}", tag="otp", bufs=2)
                        otb = []
                        for om in range(2):
                            ob = wp.tile([128, 512], bf16, name=f"ot{nt}_{om}", tag="ot", bufs=2)
                            nc.scalar.activation(ob[:], oacc[:, om, nt * 512:(nt + 1) * 512],
                                                 IDENT, bias=bias[:, 12 + om: 13 + om],
                                                 scale=1.0)
                            otb.append(ob)
                        n8 = 0
                        for om in range(2):
                            for j in range(4):
                                nc.tensor.matmul(po[:, j, om * 128:(om + 1) * 128],
                                                 otb[om][:, j * 128:(j + 1) * 128], ident[:],
                                                 is_transpose=True,
                                                 start=(n8 == 0), stop=(n8 == 7))
                                n8 += 1
                        off = wp.tile([128, 4, OUT], f32, name=f"off{nt}", tag="of", bufs=2)
                        nc.vector.tensor_copy(off[:], po[:])
                        nc.sync.dma_start(
                            out_d.ap().rearrange("(t j p) o -> t p j o", p=128, j=4)[nt],
                            off[:],
                        )

    nc.compile()
    return nc


def kernel(**inputs) -> np.ndarray:
    in_maps, K_all, devrows = _prepare(inputs)
    key = tuple(tuple(k) for k in K_all)
    nc = _CACHE.get(key)
    if nc is None:
        nc = _build(K_all)
        _CACHE.clear()
        _CACHE[key] = nc
    res = run_bass_kernel_spmd(nc, in_maps, core_ids=list(range(NCORES)))
    out = np.concatenate(
        [np.asarray(res.results[c]["out"])[devrows[c]] for c in range(NCORES)], axis=0
    )
    return out.astype(np.float32)


# revision 43
# speedup vs baseline: 1.0692x; 1.0068x over previous
"""AdaptiveAggGCN Trainium2 kernel (8 NeuronCores, data/graph-parallel).

Math: for each of G=3 graphs,
    y_i = D_in^{-1/2} A_i D_out^{-1/2} x          (sparse normalized aggregation)
    h_i = tanh(a_i * (y_i @ W_i + b_i)),  a = softmax(alphas)
    out = concat(h_i) @ W_lin + b_lin

Since row-scaling and the segment-sum commute with the dense matmuls, the
aggregation is done in the 512-wide input domain first.  Nodes are sharded
across the 8 cores by destination row (3750 each); every core gets the full
(bf16-cast) x in DRAM, gathers the source rows its edges need with
`dma_gather`, and reduces them per 128-destination block with a TensorEngine
matmul against a host-built sparse selector S (one column per edge slot,
value = a_i * rsqrt(deg_in[dst]) * rsqrt(deg_out[src])).  The dense W / W_lin
matmuls, tanh and the final output transpose all run on-chip in bf16 with
fp32 PSUM accumulation.  No collectives are needed: each core owns its output
rows and the host concatenates the 8 shards.
"""

import numpy as np
import ml_dtypes

import concourse.bacc as bacc
import concourse.bass as bass
import concourse.mybir as mybir
import concourse.tile as tile
from concourse.bass_utils import run_bass_kernel_spmd
from concourse.library_config import mlp as _mlp_lib

BF16 = ml_dtypes.bfloat16

N, IN, HID, OUT, G = 30000, 512, 512, 256, 3
NCORES = 8
SHARD = N // NCORES          # 3750 nodes per core
PAD = 4096                   # padded node count per core (8 tiles of 512)
NT = PAD // 512              # 8 node tiles for the dense matmuls
NBLK = 30                    # 128-dst blocks that can hold edges (30*128=3840)
BS = 128

_CACHE: dict = {}


def _softmax(v):
    v = np.asarray(v, np.float64)
    e = np.exp(v - v.max())
    return (e / e.sum()).astype(np.float32)


def _balance_blocks(localdeg):
    """Assign SHARD dst rows to NBLK blocks of <=128 rows, greedily equalizing
    the per-graph edge load across blocks (multi-graph LPT).  Returns
    devrow[d] = on-device row (block*128 + row) for local dst d."""
    order = np.argsort(-localdeg.sum(axis=1), kind="stable")
    loads = np.zeros((NBLK, G), np.int64)
    rows = np.zeros(NBLK, np.int64)
    blk = np.zeros(SHARD, np.int64)
    row = np.zeros(SHARD, np.int64)
    for d in order:
        cand = (loads + localdeg[d]).max(axis=1).astype(np.float64)
        cand[rows >= BS] = np.inf
        j = int(np.argmin(cand))
        blk[d] = j
        row[d] = rows[j]
        rows[j] += 1
        loads[j] += localdeg[d]
    return blk, row, blk * BS + row


def _prepare(inputs):
    """Host-side sharding/layout: returns (in_maps, K, devrows) where K[i][b]
    is the number of 128-edge chunks for graph i, dst-block b (same on all
    cores) and devrows[c][d] maps local dst d to its on-device row."""
    x = np.ascontiguousarray(np.asarray(inputs["x"], np.float32))
    xb = x.astype(BF16)
    a = _softmax(inputs["alphas"])

    ident = np.eye(128, dtype=BF16)
    # bias layout: cols 0..11 -> a_i*b_i per 128-chunk, cols 12..13 -> b_lin
    bias = np.zeros((128, 14), np.float32)
    for i in range(G):
        bi = np.asarray(inputs[f"b{i}"], np.float32) * a[i]
        bias[:, i * 4:(i + 1) * 4] = bi.reshape(4, 128).T
    bias[:, 12:14] = np.asarray(inputs["b_lin"], np.float32).reshape(2, 128).T

    # Per-core balanced dst-row -> block assignment shared by all 3 graphs.
    edges = []
    for i in range(G):
        e = np.asarray(inputs[f"edges{i}"]).astype(np.int64)
        edges.append(e)
    blk_of = []
    row_of = []
    devrows = []
    for c in range(NCORES):
        localdeg = np.zeros((SHARD, G), np.int64)
        for i in range(G):
            dst = edges[i][1]
            m = (dst >= c * SHARD) & (dst < (c + 1) * SHARD)
            localdeg[:, i] = np.bincount(dst[m] - c * SHARD, minlength=SHARD)
        blk, row, devrow = _balance_blocks(localdeg)
        blk_of.append(blk)
        row_of.append(row)
        devrows.append(devrow)

    K_all = []
    idx_arrs = [[None] * G for _ in range(NCORES)]
    s_arrs = [[None] * G for _ in range(NCORES)]
    for i in range(G):
        src, dst = edges[i][0], edges[i][1]
        deg_out = np.maximum(np.bincount(src, minlength=N), 1).astype(np.float64)
        deg_in = np.maximum(np.bincount(dst, minlength=N), 1).astype(np.float64)
        w = (a[i] / np.sqrt(deg_in[dst] * deg_out[src])).astype(np.float32)

        core = dst // SHARD
        d_loc = dst - core * SHARD
        ec = np.arange(len(dst))
        blk = np.empty(len(dst), np.int64)
        drow = np.empty(len(dst), np.int64)
        for c in range(NCORES):
            m = core == c
            blk[m] = blk_of[c][d_loc[m]]
            drow[m] = row_of[c][d_loc[m]]
        key = core * NBLK + blk
        order = np.argsort(key, kind="stable")
        src_s, w_s, drow_s = src[order], w[order], drow[order]
        cnts = np.bincount(key, minlength=NCORES * NBLK).reshape(NCORES, NBLK)
        K = np.maximum(np.ceil(cnts.max(axis=0) / BS), 1).astype(np.int64)  # [NBLK]
        K_all.append(K.tolist())
        CH = np.concatenate([[0], np.cumsum(K)])  # chunk offsets, CH[NBLK] = TK
        TK = int(CH[NBLK])
        CW = 8 * TK          # idx columns (int16, 16-wrapped)
        CS = 128 * TK        # S columns per partition

        starts = np.concatenate([[0], np.cumsum(cnts.reshape(-1))])
        for c in range(NCORES):
            idx16 = np.zeros((128, CW), np.int16)
            s_mat = np.zeros((128, CS), BF16)
            for b in range(NBLK):
                gidx = c * NBLK + b
                s0, s1 = starts[gidx], starts[gidx + 1]
                cnt = s1 - s0
                if cnt == 0:
                    continue
                sb = src_s[s0:s1]
                db = drow_s[s0:s1]
                wb = w_s[s0:s1]
                # ascending source addresses within the gather improve HBM
                # locality; slot order is free (S permutes with it)
                so = np.argsort(sb, kind="stable")
                sb, db, wb = sb[so], db[so], wb[so]
                slots = 128 * int(K[b])
                # padding slots gather row 0 with weight 0 (all slots valid,
                # so the matmul never reads unwritten SBUF)
                fl = np.zeros(slots, np.int16)
                fl[:cnt] = sb.astype(np.int16)
                # [16, 8K] block, replicated across all 128 partitions — queue q's
                # Q7 descriptor-gen CPU pair reads partitions [32q, 32q+32)
                idx16[:, 8 * CH[b]: 8 * CH[b + 1]] = np.tile(fl.reshape(-1, 16).T, (8, 1))
                j = np.arange(cnt)
                s_mat[j % 128, 128 * CH[b] + (j // 128) * 128 + db] = wb.astype(BF16)
            idx_arrs[c][i] = idx16
            s_arrs[c][i] = s_mat

    in_maps = []
    for c in range(NCORES):
        m = {
            "xb": xb,
            "ident": ident,
            "bias": bias,
            "wl": np.ascontiguousarray(np.asarray(inputs["W_lin"], np.float32)).astype(BF16),
        }
        for i in range(G):
            m[f"w{i}"] = np.ascontiguousarray(np.asarray(inputs[f"W{i}"], np.float32)).astype(BF16)
            m[f"idx{i}"] = idx_arrs[c][i]
            m[f"s{i}"] = s_arrs[c][i]
        in_maps.append(m)
    return in_maps, K_all, devrows


def _build(K_all):
    """Build the single-core (SPMD) Bass graph for chunk layout K_all."""
    f32 = mybir.dt.float32
    bf16 = mybir.dt.bfloat16
    i16 = mybir.dt.int16
    TANH = mybir.ActivationFunctionType.Tanh
    IDENT = mybir.ActivationFunctionType.Identity

    CH_all = [np.concatenate([[0], np.cumsum(K)]).astype(int) for K in K_all]
    TK = [int(CH_all[i][NBLK]) for i in range(G)]

    nc = bacc.Bacc("TRN2", target_bir_lowering=False, debug=False, num_swdge_queues=4)
    xb_d = nc.dram_tensor("xb", [N, IN], bf16, kind="ExternalInput")
    ident_d = nc.dram_tensor("ident", [128, 128], bf16, kind="ExternalInput")
    bias_d = nc.dram_tensor("bias", [128, 14], f32, kind="ExternalInput")
    wl_d = nc.dram_tensor("wl", [G * HID, OUT], bf16, kind="ExternalInput")
    w_d = [nc.dram_tensor(f"w{i}", [IN, HID], bf16, kind="ExternalInput") for i in range(G)]
    idx_d = [nc.dram_tensor(f"idx{i}", [128, 8 * TK[i]], i16, kind="ExternalInput") for i in range(G)]
    s_d = [nc.dram_tensor(f"s{i}", [128, 128 * TK[i]], bf16, kind="ExternalInput") for i in range(G)]
    out_d = nc.dram_tensor("out", [PAD, OUT], f32, kind="ExternalOutput")

    with tile.TileContext(nc) as tc:
        with (
            tc.tile_pool(name="const", bufs=1) as cp,
            tc.tile_pool(name="work", bufs=1) as wp,
            tc.tile_pool(name="ps", bufs=1, space="PSUM") as pp,
        ):
            # idx tiles first: the first gather only needs the first chunk of
            # idx0, so those DMAs go ahead of the (larger) constant loads
            idxts = []
            for i in range(G):
                idxt = wp.tile([128, 8 * TK[i]], i16, name=f"idxt{i}", tag=f"idx{i}", bufs=1)
                nchunk = 8 if i == 0 else 4
                cw = 8 * TK[i]
                step = -(-cw // nchunk)
                for c0 in range(0, cw, step):
                    nc.sync.dma_start(idxt[:, c0:min(c0 + step, cw)],
                                      idx_d[i].ap()[:, c0:min(c0 + step, cw)])
                idxts.append(idxt)
            ident = cp.tile([128, 128], bf16, name="identsb")
            nc.sync.dma_start(ident[:], ident_d.ap())
            bias = cp.tile([128, 14], f32, name="biassb")
            nc.sync.dma_start(bias[:], bias_d.ap())
            wl = cp.tile([128, 12, OUT], bf16, name="wlsb")
            nc.sync.dma_start(wl[:], wl_d.ap().rearrange("(c p) o -> p c o", p=128))
            wsb = []
            for i in range(G):
                wi = cp.tile([128, 4, HID], bf16, name=f"wsb{i}")
                nc.sync.dma_start(wi[:], w_d[i].ap().rearrange("(c p) o -> p c o", p=128))
                wsb.append(wi)
            # fp32 accumulator for the output projection, summed over graphs
            oacc = cp.tile([128, 2, PAD], f32, name="oacc")

            nc.gpsimd.load_library(_mlp_lib)

            qn = 0
            BPG = 2  # dst blocks per dma_gather (amortizes ~1us fixed overhead)
            # gather/S tile depths scale with the worst pair size so a skewed
            # edge distribution cannot blow the SBUF budget
            kt_max = max(
                sum(int(K_all[i][b]) for b in range(bp, min(bp + BPG, NBLK)))
                for i in range(G) for bp in range(0, NBLK, BPG)
            )
            g_bufs = max(3, min(8, (60 * 1024) // (kt_max * IN * 2)))
            kt_nt_max = max(
                sum(int(K_all[i][b]) for b in range(nt * 4, min(nt * 4 + 4, NBLK)))
                for i in range(G) for nt in range(NT)
            )
            s_bufs = max(2, min(4, (16 * 1024) // (kt_nt_max * 128 * 2)))
            for i in range(G):
                CH = CH_all[i]
                idxt = idxts[i]

                # One node tile (512 nodes = 4 dst blocks) at a time: aggregate,
                # transpose, then immediately run the dense W / output-projection
                # matmuls for that tile so PE has dense work while the next
                # gathers are in flight.
                for nt in range(NT):
                    ytn = wp.tile([128, 4, 512], bf16, name=f"ytn{i}_{nt}", tag="ytn", bufs=4)
                    b_lo = nt * 4
                    b_hi = min(b_lo + 4, NBLK)
                    if b_hi - b_lo < 4:
                        # node range past the last real dst block: zero-fill
                        nc.vector.memset(ytn[:, :, (b_hi - b_lo) * BS:], 0.0)
                    # one S load per node tile, one gather per pair of blocks
                    Knt = int(CH[b_hi] - CH[b_lo])
                    snt = wp.tile([128, Knt, 128], bf16, name=f"s{i}_{nt}", tag="s", bufs=s_bufs)
                    nc.sync.dma_start(
                        snt[:],
                        s_d[i].ap()[:, 128 * CH[b_lo]: 128 * CH[b_hi]]
                        .rearrange("p (k d) -> p k d", d=128),
                    )
                    for bp in range(b_lo, b_hi, BPG):
                        blks = [b for b in range(bp, min(bp + BPG, b_hi))]
                        Ks = [int(K_all[i][b]) for b in blks]
                        Ktot = sum(Ks)
                        gt = wp.tile([128, Ktot, IN], bf16, name=f"g{i}_{bp}", tag="g", bufs=g_bufs)
                        nc.gpsimd.dma_gather(
                            gt[:], xb_d.ap(), idxt[:, 8 * CH[blks[0]]: 8 * CH[blks[-1] + 1]],
                            128 * Ktot, 128 * Ktot, IN, queue_num=qn,
                            single_packet=False,
                        )
                        qn = (qn + 1) % 4
                        koff = 0
                        for bi, b in enumerate(blks):
                            Kb = Ks[bi]
                            soff = int(CH[b] - CH[b_lo])
                            py = pp.tile([128, 512], f32, name=f"py{i}_{b}", tag="acc", bufs=3)
                            for k in range(Kb):
                                nc.tensor.matmul(py[:], snt[:, soff + k, :], gt[:, koff + k, :],
                                                 start=(k == 0), stop=(k == Kb - 1))
                            koff += Kb
                            ysb = wp.tile([128, 512], bf16, name=f"y{i}_{b}", tag="ysb", bufs=2)
                            nc.vector.tensor_copy(ysb[:], py[:])
                            pyt = pp.tile([128, 4, 128], bf16, name=f"pyt{i}_{b}", tag="ytp", bufs=2)
                            for fc in range(4):
                                nc.tensor.matmul(pyt[:, fc, :], ysb[:, fc * 128:(fc + 1) * 128],
                                                 ident[:], is_transpose=True,
                                                 start=(fc == 0), stop=(fc == 3))
                            nc.vector.tensor_copy(ytn[:, :, (b - b_lo) * BS:(b - b_lo + 1) * BS],
                                                  pyt[:])

                    htile = wp.tile([128, 4, 512], bf16, name=f"ht{i}_{nt}", tag="ht", bufs=4)
                    for of in range(4):
                        ph = pp.tile([128, 512], f32, name=f"ph{i}_{nt}_{of}", tag="acc", bufs=3)
                        for c4 in range(4):
                            nc.tensor.matmul(ph[:], wsb[i][:, c4, of * 128:(of + 1) * 128],
                                             ytn[:, c4, :],
                                             start=(c4 == 0), stop=(c4 == 3))
                        nc.scalar.activation(htile[:, of, :], ph[:],
                                             TANH, bias=bias[:, i * 4 + of: i * 4 + of + 1],
                                             scale=1.0)
                    for om in range(2):
                        pacc = pp.tile([128, 512], f32, name=f"pa{i}_{nt}_{om}", tag="acc", bufs=3)
                        for hf in range(4):
                            nc.tensor.matmul(pacc[:], wl[:, i * 4 + hf, om * 128:(om + 1) * 128],
                                             htile[:, hf, :],
                                             start=(hf == 0), stop=(hf == 3))
                        dst = oacc[:, om, nt * 512:(nt + 1) * 512]
                        if i == 0:
                            nc.vector.tensor_copy(dst, pacc[:])
                        else:
                            nc.vector.tensor_add(dst, dst, pacc[:])

                    if i == G - 1:
                        # output epilogue for this node tile: +b_lin, transpose
                        # back to node-major, DMA out
                        po = pp.tile([128, 4, OUT], bf16, name=f"po{nt}", tag="otp", bufs=2)
                        otb = []
                        for om in range(2):
                            ob = wp.tile([128, 512], bf16, name=f"ot{nt}_{om}", tag="ot", bufs=2)
                            nc.scalar.activation(ob[:], oacc[:, om, nt * 512:(nt + 1) * 512],
                                                 IDENT, bias=bias[:, 12 + om: 13 + om],
                                                 scale=1.0)
                            otb.append(ob)
                        n8 = 0
                        for om in range(2):
                            for j in range(4):
                                nc.tensor.matmul(po[:, j, om * 128:(om + 1) * 128],
                                                 otb[om][:, j * 128:(j + 1) * 128], ident[:],
                                                 is_transpose=True,
                                                 start=(n8 == 0), stop=(n8 == 7))
                                n8 += 1
                        off = wp.tile([128, 4, OUT], f32, name=f"off{nt}", tag="of", bufs=2)
                        nc.vector.tensor_copy(off[:], po[:])
                        nc.sync.dma_start(
                            out_d.ap().rearrange("(t j p) o -> t p j o", p=128, j=4)[nt],
                            off[:],
                        )

    nc.compile()
    return nc


def kernel(**inputs) -> np.ndarray:
    in_maps, K_all, devrows = _prepare(inputs)
    key = tuple(tuple(k) for k in K_all)
    nc = _CACHE.get(key)
    if nc is None:
        nc = _build(K_all)
        _CACHE.clear()
        _CACHE[key] = nc
    res = run_bass_kernel_spmd(nc, in_maps, core_ids=list(range(NCORES)))
    out = np.concatenate(
        [np.asarray(res.results[c]["out"])[devrows[c]] for c in range(NCORES)], axis=0
    )
    return out.astype(np.float32)


# revision 44
# speedup vs baseline: 1.0720x; 1.0027x over previous
"""AdaptiveAggGCN Trainium2 kernel (8 NeuronCores, data/graph-parallel).

Math: for each of G=3 graphs,
    y_i = D_in^{-1/2} A_i D_out^{-1/2} x          (sparse normalized aggregation)
    h_i = tanh(a_i * (y_i @ W_i + b_i)),  a = softmax(alphas)
    out = concat(h_i) @ W_lin + b_lin

Since row-scaling and the segment-sum commute with the dense matmuls, the
aggregation is done in the 512-wide input domain first.  Nodes are sharded
across the 8 cores by destination row (3750 each); every core gets the full
(bf16-cast) x in DRAM, gathers the source rows its edges need with
`dma_gather`, and reduces them per 128-destination block with a TensorEngine
matmul against a host-built sparse selector S (one column per edge slot,
value = a_i * rsqrt(deg_in[dst]) * rsqrt(deg_out[src])).  The dense W / W_lin
matmuls, tanh and the final output transpose all run on-chip in bf16 with
fp32 PSUM accumulation.  No collectives are needed: each core owns its output
rows and the host concatenates the 8 shards.
"""

import numpy as np
import ml_dtypes

import concourse.bacc as bacc
import concourse.mybir as mybir
import concourse.tile as tile
from concourse.bass_utils import run_bass_kernel_spmd
from concourse.library_config import mlp as _mlp_lib

BF16 = ml_dtypes.bfloat16

N, IN, HID, OUT, G = 30000, 512, 512, 256, 3
NCORES = 8
SHARD = N // NCORES          # 3750 nodes per core
PAD = 4096                   # padded node count per core (8 tiles of 512)
NT = PAD // 512              # 8 node tiles for the dense matmuls
NBLK = 30                    # 128-dst blocks that can hold edges (30*128=3840)
BS = 128

_CACHE: dict = {}


def _softmax(v):
    v = np.asarray(v, np.float64)
    e = np.exp(v - v.max())
    return (e / e.sum()).astype(np.float32)


def _balance_blocks(localdeg):
    """Assign SHARD dst rows to NBLK blocks of <=128 rows, greedily equalizing
    the per-graph edge load across blocks (multi-graph LPT).  Returns
    devrow[d] = on-device row (block*128 + row) for local dst d."""
    order = np.argsort(-localdeg.sum(axis=1), kind="stable")
    loads = np.zeros((NBLK, G), np.int64)
    rows = np.zeros(NBLK, np.int64)
    blk = np.zeros(SHARD, np.int64)
    row = np.zeros(SHARD, np.int64)
    for d in order:
        cand = (loads + localdeg[d]).max(axis=1).astype(np.float64)
        cand[rows >= BS] = np.inf
        j = int(np.argmin(cand))
        blk[d] = j
        row[d] = rows[j]
        rows[j] += 1
        loads[j] += localdeg[d]
    return blk, row, blk * BS + row


def _prepare(inputs):
    """Host-side sharding/layout: returns (in_maps, K, devrows) where K[i][b]
    is the number of 128-edge chunks for graph i, dst-block b (same on all
    cores) and devrows[c][d] maps local dst d to its on-device row."""
    x = np.ascontiguousarray(np.asarray(inputs["x"], np.float32))
    xb = x.astype(BF16)
    a = _softmax(inputs["alphas"])

    ident = np.eye(128, dtype=BF16)
    # bias layout: cols 0..11 -> a_i*b_i per 128-chunk, cols 12..13 -> b_lin
    bias = np.zeros((128, 14), np.float32)
    for i in range(G):
        bi = np.asarray(inputs[f"b{i}"], np.float32) * a[i]
        bias[:, i * 4:(i + 1) * 4] = bi.reshape(4, 128).T
    bias[:, 12:14] = np.asarray(inputs["b_lin"], np.float32).reshape(2, 128).T

    # Per-core balanced dst-row -> block assignment shared by all 3 graphs.
    edges = []
    for i in range(G):
        e = np.asarray(inputs[f"edges{i}"]).astype(np.int64)
        edges.append(e)
    blk_of = []
    row_of = []
    devrows = []
    for c in range(NCORES):
        localdeg = np.zeros((SHARD, G), np.int64)
        for i in range(G):
            dst = edges[i][1]
            m = (dst >= c * SHARD) & (dst < (c + 1) * SHARD)
            localdeg[:, i] = np.bincount(dst[m] - c * SHARD, minlength=SHARD)
        blk, row, devrow = _balance_blocks(localdeg)
        blk_of.append(blk)
        row_of.append(row)
        devrows.append(devrow)

    K_all = []
    idx_arrs = [[None] * G for _ in range(NCORES)]
    s_arrs = [[None] * G for _ in range(NCORES)]
    for i in range(G):
        src, dst = edges[i][0], edges[i][1]
        deg_out = np.maximum(np.bincount(src, minlength=N), 1).astype(np.float64)
        deg_in = np.maximum(np.bincount(dst, minlength=N), 1).astype(np.float64)
        w = (a[i] / np.sqrt(deg_in[dst] * deg_out[src])).astype(np.float32)

        core = dst // SHARD
        d_loc = dst - core * SHARD
        blk = np.empty(len(dst), np.int64)
        drow = np.empty(len(dst), np.int64)
        for c in range(NCORES):
            m = core == c
            blk[m] = blk_of[c][d_loc[m]]
            drow[m] = row_of[c][d_loc[m]]
        key = core * NBLK + blk
        order = np.argsort(key, kind="stable")
        src_s, w_s, drow_s = src[order], w[order], drow[order]
        cnts = np.bincount(key, minlength=NCORES * NBLK).reshape(NCORES, NBLK)
        K = np.maximum(np.ceil(cnts.max(axis=0) / BS), 1).astype(np.int64)  # [NBLK]
        K_all.append(K.tolist())
        CH = np.concatenate([[0], np.cumsum(K)])  # chunk offsets, CH[NBLK] = TK
        TK = int(CH[NBLK])
        CW = 8 * TK          # idx columns (int16, 16-wrapped)
        CS = 128 * TK        # S columns per partition

        starts = np.concatenate([[0], np.cumsum(cnts.reshape(-1))])
        for c in range(NCORES):
            idx16 = np.zeros((128, CW), np.int16)
            s_mat = np.zeros((128, CS), BF16)
            for b in range(NBLK):
                gidx = c * NBLK + b
                s0, s1 = starts[gidx], starts[gidx + 1]
                cnt = s1 - s0
                if cnt == 0:
                    continue
                sb = src_s[s0:s1]
                db = drow_s[s0:s1]
                wb = w_s[s0:s1]
                # ascending source addresses within the gather improve HBM
                # locality; slot order is free (S permutes with it)
                so = np.argsort(sb, kind="stable")
                sb, db, wb = sb[so], db[so], wb[so]
                slots = 128 * int(K[b])
                # padding slots gather row 0 with weight 0 (all slots valid,
                # so the matmul never reads unwritten SBUF)
                fl = np.zeros(slots, np.int16)
                fl[:cnt] = sb.astype(np.int16)
                # [16, 8K] block, replicated across all 128 partitions — queue q's
                # Q7 descriptor-gen CPU pair reads partitions [32q, 32q+32)
                idx16[:, 8 * CH[b]: 8 * CH[b + 1]] = np.tile(fl.reshape(-1, 16).T, (8, 1))
                j = np.arange(cnt)
                s_mat[j % 128, 128 * CH[b] + (j // 128) * 128 + db] = wb.astype(BF16)
            idx_arrs[c][i] = idx16
            s_arrs[c][i] = s_mat

    in_maps = []
    for c in range(NCORES):
        m = {
            "xb": xb,
            "ident": ident,
            "bias": bias,
            "wl": np.ascontiguousarray(np.asarray(inputs["W_lin"], np.float32)).astype(BF16),
        }
        for i in range(G):
            m[f"w{i}"] = np.ascontiguousarray(np.asarray(inputs[f"W{i}"], np.float32)).astype(BF16)
            m[f"idx{i}"] = idx_arrs[c][i]
            m[f"s{i}"] = s_arrs[c][i]
        in_maps.append(m)
    return in_maps, K_all, devrows


def _build(K_all):
    """Build the single-core (SPMD) Bass graph for chunk layout K_all."""
    f32 = mybir.dt.float32
    bf16 = mybir.dt.bfloat16
    i16 = mybir.dt.int16
    TANH = mybir.ActivationFunctionType.Tanh
    IDENT = mybir.ActivationFunctionType.Identity

    CH_all = [np.concatenate([[0], np.cumsum(K)]).astype(int) for K in K_all]
    TK = [int(CH_all[i][NBLK]) for i in range(G)]

    nc = bacc.Bacc("TRN2", target_bir_lowering=False, debug=False, num_swdge_queues=4)
    xb_d = nc.dram_tensor("xb", [N, IN], bf16, kind="ExternalInput")
    ident_d = nc.dram_tensor("ident", [128, 128], bf16, kind="ExternalInput")
    bias_d = nc.dram_tensor("bias", [128, 14], f32, kind="ExternalInput")
    wl_d = nc.dram_tensor("wl", [G * HID, OUT], bf16, kind="ExternalInput")
    w_d = [nc.dram_tensor(f"w{i}", [IN, HID], bf16, kind="ExternalInput") for i in range(G)]
    idx_d = [nc.dram_tensor(f"idx{i}", [128, 8 * TK[i]], i16, kind="ExternalInput") for i in range(G)]
    s_d = [nc.dram_tensor(f"s{i}", [128, 128 * TK[i]], bf16, kind="ExternalInput") for i in range(G)]
    out_d = nc.dram_tensor("out", [PAD, OUT], f32, kind="ExternalOutput")

    with tile.TileContext(nc) as tc:
        with (
            tc.tile_pool(name="const", bufs=1) as cp,
            tc.tile_pool(name="work", bufs=1) as wp,
            tc.tile_pool(name="ps", bufs=1, space="PSUM") as pp,
        ):
            # idx tiles first: the first gather only needs the first chunk of
            # idx0, so those DMAs go ahead of the (larger) constant loads
            idxts = []
            for i in range(G):
                idxt = wp.tile([128, 8 * TK[i]], i16, name=f"idxt{i}", tag=f"idx{i}", bufs=1)
                nchunk = 8 if i == 0 else 4
                cw = 8 * TK[i]
                step = -(-cw // nchunk)
                for c0 in range(0, cw, step):
                    nc.sync.dma_start(idxt[:, c0:min(c0 + step, cw)],
                                      idx_d[i].ap()[:, c0:min(c0 + step, cw)])
                idxts.append(idxt)
            ident = cp.tile([128, 128], bf16, name="identsb")
            nc.sync.dma_start(ident[:], ident_d.ap())
            bias = cp.tile([128, 14], f32, name="biassb")
            nc.sync.dma_start(bias[:], bias_d.ap())
            wl = cp.tile([128, 12, OUT], bf16, name="wlsb")
            nc.sync.dma_start(wl[:], wl_d.ap().rearrange("(c p) o -> p c o", p=128))
            wsb = []
            for i in range(G):
                wi = cp.tile([128, 4, HID], bf16, name=f"wsb{i}")
                nc.sync.dma_start(wi[:], w_d[i].ap().rearrange("(c p) o -> p c o", p=128))
                wsb.append(wi)
            # fp32 accumulator for the output projection, summed over graphs
            oacc = cp.tile([128, 2, PAD], f32, name="oacc")

            nc.gpsimd.load_library(_mlp_lib)

            qn = 0
            BPG = 2  # dst blocks per dma_gather (amortizes ~1us fixed overhead)
            # gather/S tile depths scale with the worst pair size so a skewed
            # edge distribution cannot blow the SBUF budget
            kt_max = max(
                sum(int(K_all[i][b]) for b in range(bp, min(bp + BPG, NBLK)))
                for i in range(G) for bp in range(0, NBLK, BPG)
            )
            g_bufs = max(3, min(8, (60 * 1024) // (kt_max * IN * 2)))
            kt_nt_max = max(
                sum(int(K_all[i][b]) for b in range(nt * 4, min(nt * 4 + 4, NBLK)))
                for i in range(G) for nt in range(NT)
            )
            s_bufs = max(2, min(4, (16 * 1024) // (kt_nt_max * 128 * 2)))
            for i in range(G):
                CH = CH_all[i]
                idxt = idxts[i]

                # One node tile (512 nodes = 4 dst blocks) at a time: aggregate,
                # transpose, then immediately run the dense W / output-projection
                # matmuls for that tile so PE has dense work while the next
                # gathers are in flight.
                for nt in range(NT):
                    ytn = wp.tile([128, 4, 512], bf16, name=f"ytn{i}_{nt}", tag="ytn", bufs=4)
                    b_lo = nt * 4
                    b_hi = min(b_lo + 4, NBLK)
                    if b_hi - b_lo < 4:
                        # node range past the last real dst block: zero-fill
                        nc.vector.memset(ytn[:, :, (b_hi - b_lo) * BS:], 0.0)
                    # one S load per node tile, one gather per pair of blocks
                    Knt = int(CH[b_hi] - CH[b_lo])
                    snt = wp.tile([128, Knt, 128], bf16, name=f"s{i}_{nt}", tag="s", bufs=s_bufs)
                    nc.sync.dma_start(
                        snt[:],
                        s_d[i].ap()[:, 128 * CH[b_lo]: 128 * CH[b_hi]]
                        .rearrange("p (k d) -> p k d", d=128),
                    )
                    for bp in range(b_lo, b_hi, BPG):
                        blks = [b for b in range(bp, min(bp + BPG, b_hi))]
                        Ks = [int(K_all[i][b]) for b in blks]
                        Ktot = sum(Ks)
                        gt = wp.tile([128, Ktot, IN], bf16, name=f"g{i}_{bp}", tag="g", bufs=g_bufs)
                        nc.gpsimd.dma_gather(
                            gt[:], xb_d.ap(), idxt[:, 8 * CH[blks[0]]: 8 * CH[blks[-1] + 1]],
                            128 * Ktot, 128 * Ktot, IN, queue_num=qn,
                            single_packet=False,
                        )
                        qn = (qn + 1) % 4
                        koff = 0
                        for bi, b in enumerate(blks):
                            Kb = Ks[bi]
                            soff = int(CH[b] - CH[b_lo])
                            py = pp.tile([128, 512], f32, name=f"py{i}_{b}", tag="acc", bufs=3)
                            for k in range(Kb):
                                nc.tensor.matmul(py[:], snt[:, soff + k, :], gt[:, koff + k, :],
                                                 start=(k == 0), stop=(k == Kb - 1))
                            koff += Kb
                            ysb = wp.tile([128, 512], bf16, name=f"y{i}_{b}", tag="ysb", bufs=2)
                            nc.vector.tensor_copy(ysb[:], py[:])
                            pyt = pp.tile([128, 4, 128], bf16, name=f"pyt{i}_{b}", tag="ytp", bufs=2)
                            for fc in range(4):
                                nc.tensor.matmul(pyt[:, fc, :], ysb[:, fc * 128:(fc + 1) * 128],
                                                 ident[:], is_transpose=True,
                                                 start=(fc == 0), stop=(fc == 3))
                            nc.vector.tensor_copy(ytn[:, :, (b - b_lo) * BS:(b - b_lo + 1) * BS],
                                                  pyt[:])

                    htile = wp.tile([128, 4, 512], bf16, name=f"ht{i}_{nt}", tag="ht", bufs=4)
                    for of in range(4):
                        ph = pp.tile([128, 512], f32, name=f"ph{i}_{nt}_{of}", tag="acc", bufs=3)
                        for c4 in range(4):
                            nc.tensor.matmul(ph[:], wsb[i][:, c4, of * 128:(of + 1) * 128],
                                             ytn[:, c4, :],
                                             start=(c4 == 0), stop=(c4 == 3))
                        nc.scalar.activation(htile[:, of, :], ph[:],
                                             TANH, bias=bias[:, i * 4 + of: i * 4 + of + 1],
                                             scale=1.0)
                    for om in range(2):
                        pacc = pp.tile([128, 512], f32, name=f"pa{i}_{nt}_{om}", tag="acc", bufs=3)
                        for hf in range(4):
                            nc.tensor.matmul(pacc[:], wl[:, i * 4 + hf, om * 128:(om + 1) * 128],
                                             htile[:, hf, :],
                                             start=(hf == 0), stop=(hf == 3))
                        dst = oacc[:, om, nt * 512:(nt + 1) * 512]
                        if i == 0:
                            nc.vector.tensor_copy(dst, pacc[:])
                        else:
                            nc.vector.tensor_add(dst, dst, pacc[:])

                    if i == G - 1:
                        # output epilogue for this node tile: +b_lin, transpose
                        # back to node-major, DMA out
                        po = pp.tile([128, 4, OUT], bf16, name=f"po{nt}", tag="otp", bufs=2)
                        otb = []
                        for om in range(2):
                            ob = wp.tile([128, 512], bf16, name=f"ot{nt}_{om}", tag="ot", bufs=2)
                            nc.scalar.activation(ob[:], oacc[:, om, nt * 512:(nt + 1) * 512],
                                                 IDENT, bias=bias[:, 12 + om: 13 + om],
                                                 scale=1.0)
                            otb.append(ob)
                        n8 = 0
                        for om in range(2):
                            for j in range(4):
                                nc.tensor.matmul(po[:, j, om * 128:(om + 1) * 128],
                                                 otb[om][:, j * 128:(j + 1) * 128], ident[:],
                                                 is_transpose=True,
                                                 start=(n8 == 0), stop=(n8 == 7))
                                n8 += 1
                        off = wp.tile([128, 4, OUT], f32, name=f"off{nt}", tag="of", bufs=2)
                        nc.vector.tensor_copy(off[:], po[:])
                        nc.sync.dma_start(
                            out_d.ap().rearrange("(t j p) o -> t p j o", p=128, j=4)[nt],
                            off[:],
                        )

    nc.compile()
    return nc


def kernel(**inputs) -> np.ndarray:
    in_maps, K_all, devrows = _prepare(inputs)
    key = tuple(tuple(k) for k in K_all)
    nc = _CACHE.get(key)
    if nc is None:
        nc = _build(K_all)
        _CACHE.clear()
        _CACHE[key] = nc
    res = run_bass_kernel_spmd(nc, in_maps, core_ids=list(range(NCORES)))
    out = np.concatenate(
        [np.asarray(res.results[c]["out"])[devrows[c]] for c in range(NCORES)], axis=0
    )
    return out.astype(np.float32)


# revision 46
# speedup vs baseline: 1.1016x; 1.0276x over previous
"""AdaptiveAggGCN Trainium2 kernel (8 NeuronCores, data/graph-parallel).

Math: for each of G=3 graphs,
    y_i = D_in^{-1/2} A_i D_out^{-1/2} x          (sparse normalized aggregation)
    h_i = tanh(a_i * (y_i @ W_i + b_i)),  a = softmax(alphas)
    out = concat(h_i) @ W_lin + b_lin

Since row-scaling and the segment-sum commute with the dense matmuls, the
aggregation is done in the 512-wide input domain first.  Nodes are sharded
across the 8 cores by destination row (3750 each); every core gets the full
(bf16-cast) x in DRAM, gathers the source rows its edges need with
`dma_gather`, and reduces them per 128-destination block with a TensorEngine
matmul against a host-built sparse selector S (one column per edge slot,
value = a_i * rsqrt(deg_in[dst]) * rsqrt(deg_out[src])).  The dense W / W_lin
matmuls, tanh and the final output transpose all run on-chip in bf16 with
fp32 PSUM accumulation.  No collectives are needed: each core owns its output
rows and the host concatenates the 8 shards.
"""

import numpy as np
import ml_dtypes

import concourse.bacc as bacc
import concourse.mybir as mybir
import concourse.tile as tile
from concourse.bass_utils import run_bass_kernel_spmd
from concourse.library_config import mlp as _mlp_lib

BF16 = ml_dtypes.bfloat16

N, IN, HID, OUT, G = 30000, 512, 512, 256, 3
NCORES = 8
SHARD = N // NCORES          # 3750 nodes per core
PAD = 4096                   # padded node count per core (8 tiles of 512)
NT = PAD // 512              # 8 node tiles for the dense matmuls
NBLK = 30                    # 128-dst blocks that can hold edges (30*128=3840)
BS = 128

_CACHE: dict = {}


def _softmax(v):
    v = np.asarray(v, np.float64)
    e = np.exp(v - v.max())
    return (e / e.sum()).astype(np.float32)


def _balance_blocks(localdeg):
    """Assign SHARD dst rows to NBLK blocks of <=128 rows, greedily equalizing
    the per-graph edge load across blocks (multi-graph LPT).  Returns
    devrow[d] = on-device row (block*128 + row) for local dst d."""
    order = np.argsort(-localdeg.sum(axis=1), kind="stable")
    loads = np.zeros((NBLK, G), np.int64)
    rows = np.zeros(NBLK, np.int64)
    blk = np.zeros(SHARD, np.int64)
    row = np.zeros(SHARD, np.int64)
    for d in order:
        cand = (loads + localdeg[d]).max(axis=1).astype(np.float64)
        cand[rows >= BS] = np.inf
        j = int(np.argmin(cand))
        blk[d] = j
        row[d] = rows[j]
        rows[j] += 1
        loads[j] += localdeg[d]
    return blk, row, blk * BS + row


def _prepare(inputs):
    """Host-side sharding/layout: returns (in_maps, K, devrows) where K[i][b]
    is the number of 128-edge chunks for graph i, dst-block b (same on all
    cores) and devrows[c][d] maps local dst d to its on-device row."""
    x = np.ascontiguousarray(np.asarray(inputs["x"], np.float32))
    xb = x.astype(BF16)
    a = _softmax(inputs["alphas"])

    ident = np.eye(128, dtype=BF16)
    # bias layout: cols 0..11 -> a_i*b_i per 128-chunk, cols 12..13 -> b_lin
    bias = np.zeros((128, 14), np.float32)
    for i in range(G):
        bi = np.asarray(inputs[f"b{i}"], np.float32) * a[i]
        bias[:, i * 4:(i + 1) * 4] = bi.reshape(4, 128).T
    bias[:, 12:14] = np.asarray(inputs["b_lin"], np.float32).reshape(2, 128).T

    # Per-core balanced dst-row -> block assignment shared by all 3 graphs.
    edges = []
    for i in range(G):
        e = np.asarray(inputs[f"edges{i}"]).astype(np.int64)
        edges.append(e)
    blk_of = []
    row_of = []
    devrows = []
    for c in range(NCORES):
        localdeg = np.zeros((SHARD, G), np.int64)
        for i in range(G):
            dst = edges[i][1]
            m = (dst >= c * SHARD) & (dst < (c + 1) * SHARD)
            localdeg[:, i] = np.bincount(dst[m] - c * SHARD, minlength=SHARD)
        blk, row, devrow = _balance_blocks(localdeg)
        blk_of.append(blk)
        row_of.append(row)
        devrows.append(devrow)

    K_all = []
    idx_arrs = [[None] * G for _ in range(NCORES)]
    s_arrs = [[None] * G for _ in range(NCORES)]
    for i in range(G):
        src, dst = edges[i][0], edges[i][1]
        deg_out = np.maximum(np.bincount(src, minlength=N), 1).astype(np.float64)
        deg_in = np.maximum(np.bincount(dst, minlength=N), 1).astype(np.float64)
        w = (a[i] / np.sqrt(deg_in[dst] * deg_out[src])).astype(np.float32)

        core = dst // SHARD
        d_loc = dst - core * SHARD
        blk = np.empty(len(dst), np.int64)
        drow = np.empty(len(dst), np.int64)
        for c in range(NCORES):
            m = core == c
            blk[m] = blk_of[c][d_loc[m]]
            drow[m] = row_of[c][d_loc[m]]
        key = core * NBLK + blk
        order = np.argsort(key, kind="stable")
        src_s, w_s, drow_s = src[order], w[order], drow[order]
        cnts = np.bincount(key, minlength=NCORES * NBLK).reshape(NCORES, NBLK)
        K = np.maximum(np.ceil(cnts.max(axis=0) / BS), 1).astype(np.int64)  # [NBLK]
        K_all.append(K.tolist())
        CH = np.concatenate([[0], np.cumsum(K)])  # chunk offsets, CH[NBLK] = TK
        TK = int(CH[NBLK])
        CW = 8 * TK          # idx columns (int16, 16-wrapped)
        CS = 128 * TK        # S columns per partition

        starts = np.concatenate([[0], np.cumsum(cnts.reshape(-1))])
        for c in range(NCORES):
            idx16 = np.zeros((128, CW), np.int16)
            s_mat = np.zeros((128, CS), BF16)
            for b in range(NBLK):
                gidx = c * NBLK + b
                s0, s1 = starts[gidx], starts[gidx + 1]
                cnt = s1 - s0
                if cnt == 0:
                    continue
                sb = src_s[s0:s1]
                db = drow_s[s0:s1]
                wb = w_s[s0:s1]
                # ascending source addresses within the gather improve HBM
                # locality; slot order is free (S permutes with it)
                so = np.argsort(sb, kind="stable")
                sb, db, wb = sb[so], db[so], wb[so]
                slots = 128 * int(K[b])
                # padding slots gather row 0 with weight 0 (all slots valid,
                # so the matmul never reads unwritten SBUF)
                fl = np.zeros(slots, np.int16)
                fl[:cnt] = sb.astype(np.int16)
                # [16, 8K] block, replicated across all 128 partitions — queue q's
                # Q7 descriptor-gen CPU pair reads partitions [32q, 32q+32)
                idx16[:, 8 * CH[b]: 8 * CH[b + 1]] = np.tile(fl.reshape(-1, 16).T, (8, 1))
                j = np.arange(cnt)
                s_mat[j % 128, 128 * CH[b] + (j // 128) * 128 + db] = wb.astype(BF16)
            idx_arrs[c][i] = idx16
            s_arrs[c][i] = s_mat

    in_maps = []
    for c in range(NCORES):
        m = {
            "xb": xb,
            "ident": ident,
            "bias": bias,
            "wl": np.ascontiguousarray(np.asarray(inputs["W_lin"], np.float32)).astype(BF16),
        }
        for i in range(G):
            m[f"w{i}"] = np.ascontiguousarray(np.asarray(inputs[f"W{i}"], np.float32)).astype(BF16)
            m[f"idx{i}"] = idx_arrs[c][i]
            m[f"s{i}"] = s_arrs[c][i]
        in_maps.append(m)
    return in_maps, K_all, devrows


def _build(K_all):
    """Build the single-core (SPMD) Bass graph for chunk layout K_all."""
    f32 = mybir.dt.float32
    bf16 = mybir.dt.bfloat16
    i16 = mybir.dt.int16
    TANH = mybir.ActivationFunctionType.Tanh
    IDENT = mybir.ActivationFunctionType.Identity

    CH_all = [np.concatenate([[0], np.cumsum(K)]).astype(int) for K in K_all]
    TK = [int(CH_all[i][NBLK]) for i in range(G)]

    nc = bacc.Bacc("TRN2", target_bir_lowering=False, debug=False, num_swdge_queues=4)
    xb_d = nc.dram_tensor("xb", [N, IN], bf16, kind="ExternalInput")
    ident_d = nc.dram_tensor("ident", [128, 128], bf16, kind="ExternalInput")
    bias_d = nc.dram_tensor("bias", [128, 14], f32, kind="ExternalInput")
    wl_d = nc.dram_tensor("wl", [G * HID, OUT], bf16, kind="ExternalInput")
    w_d = [nc.dram_tensor(f"w{i}", [IN, HID], bf16, kind="ExternalInput") for i in range(G)]
    idx_d = [nc.dram_tensor(f"idx{i}", [128, 8 * TK[i]], i16, kind="ExternalInput") for i in range(G)]
    s_d = [nc.dram_tensor(f"s{i}", [128, 128 * TK[i]], bf16, kind="ExternalInput") for i in range(G)]
    out_d = nc.dram_tensor("out", [PAD, OUT], f32, kind="ExternalOutput")

    with tile.TileContext(nc) as tc:
        with (
            tc.tile_pool(name="const", bufs=1) as cp,
            tc.tile_pool(name="work", bufs=1) as wp,
            tc.tile_pool(name="ps", bufs=1, space="PSUM") as pp,
        ):
            ident = cp.tile([128, 128], bf16, name="identsb")
            nc.sync.dma_start(ident[:], ident_d.ap())
            bias = cp.tile([128, 14], f32, name="biassb")
            nc.sync.dma_start(bias[:], bias_d.ap())
            wl = cp.tile([128, 12, OUT], bf16, name="wlsb")
            nc.sync.dma_start(wl[:], wl_d.ap().rearrange("(c p) o -> p c o", p=128))
            wsb = []
            for i in range(G):
                wi = cp.tile([128, 4, HID], bf16, name=f"wsb{i}")
                nc.sync.dma_start(wi[:], w_d[i].ap().rearrange("(c p) o -> p c o", p=128))
                wsb.append(wi)
            # fp32 accumulator for the output projection, summed over graphs
            oacc = cp.tile([128, 2, PAD], f32, name="oacc")

            nc.gpsimd.load_library(_mlp_lib)

            qn = 0
            BPG = 2  # dst blocks per dma_gather (amortizes ~1us fixed overhead)
            # gather/S tile depths scale with the worst pair size so a skewed
            # edge distribution cannot blow the SBUF budget
            kt_max = max(
                sum(int(K_all[i][b]) for b in range(bp, min(bp + BPG, NBLK)))
                for i in range(G) for bp in range(0, NBLK, BPG)
            )
            g_bufs = max(3, min(8, (70 * 1024) // (kt_max * IN * 2)))
            kt_nt_max = max(
                sum(int(K_all[i][b]) for b in range(nt * 4, min(nt * 4 + 4, NBLK)))
                for i in range(G) for nt in range(NT)
            )
            s_bufs = max(2, min(4, (16 * 1024) // (kt_nt_max * 128 * 2)))
            for i in range(G):
                CH = CH_all[i]

                # One node tile (512 nodes = 4 dst blocks) at a time: aggregate,
                # transpose, then immediately run the dense W / output-projection
                # matmuls for that tile so PE has dense work while the next
                # gathers are in flight.
                for nt in range(NT):
                    ytn = wp.tile([128, 4, 512], bf16, name=f"ytn{i}_{nt}", tag="ytn", bufs=4)
                    b_lo = nt * 4
                    b_hi = min(b_lo + 4, NBLK)
                    if b_hi - b_lo < 4:
                        # node range past the last real dst block: zero-fill
                        nc.vector.memset(ytn[:, :, (b_hi - b_lo) * BS:], 0.0)
                    # per-tile idx + S loads: small tiles keep Tile's whole-tile
                    # dependencies off the gather critical path
                    Knt = int(CH[b_hi] - CH[b_lo])
                    idxt = wp.tile([128, 8 * Knt], i16, name=f"idx{i}_{nt}", tag="idx", bufs=4)
                    nc.sync.dma_start(idxt[:], idx_d[i].ap()[:, 8 * CH[b_lo]: 8 * CH[b_hi]])
                    snt = wp.tile([128, Knt, 128], bf16, name=f"s{i}_{nt}", tag="s", bufs=s_bufs)
                    nc.sync.dma_start(
                        snt[:],
                        s_d[i].ap()[:, 128 * CH[b_lo]: 128 * CH[b_hi]]
                        .rearrange("p (k d) -> p k d", d=128),
                    )
                    for bp in range(b_lo, b_hi, BPG):
                        blks = [b for b in range(bp, min(bp + BPG, b_hi))]
                        Ks = [int(K_all[i][b]) for b in blks]
                        Ktot = sum(Ks)
                        gt = wp.tile([128, Ktot, IN], bf16, name=f"g{i}_{bp}", tag="g", bufs=g_bufs)
                        nc.gpsimd.dma_gather(
                            gt[:], xb_d.ap(),
                            idxt[:, 8 * (CH[blks[0]] - CH[b_lo]): 8 * (CH[blks[-1] + 1] - CH[b_lo])],
                            128 * Ktot, 128 * Ktot, IN, queue_num=qn,
                            single_packet=False,
                        )
                        qn = (qn + 1) % 4
                        koff = 0
                        for bi, b in enumerate(blks):
                            Kb = Ks[bi]
                            soff = int(CH[b] - CH[b_lo])
                            py = pp.tile([128, 512], f32, name=f"py{i}_{b}", tag="acc", bufs=3)
                            for k in range(Kb):
                                nc.tensor.matmul(py[:], snt[:, soff + k, :], gt[:, koff + k, :],
                                                 start=(k == 0), stop=(k == Kb - 1))
                            koff += Kb
                            ysb = wp.tile([128, 512], bf16, name=f"y{i}_{b}", tag="ysb", bufs=2)
                            nc.vector.tensor_copy(ysb[:], py[:])
                            pyt = pp.tile([128, 4, 128], bf16, name=f"pyt{i}_{b}", tag="ytp", bufs=2)
                            for fc in range(4):
                                nc.tensor.matmul(pyt[:, fc, :], ysb[:, fc * 128:(fc + 1) * 128],
                                                 ident[:], is_transpose=True,
                                                 start=(fc == 0), stop=(fc == 3))
                            nc.vector.tensor_copy(ytn[:, :, (b - b_lo) * BS:(b - b_lo + 1) * BS],
                                                  pyt[:])

                    htile = wp.tile([128, 4, 512], bf16, name=f"ht{i}_{nt}", tag="ht", bufs=4)
                    for of in range(4):
                        ph = pp.tile([128, 512], f32, name=f"ph{i}_{nt}_{of}", tag="acc", bufs=3)
                        for c4 in range(4):
                            nc.tensor.matmul(ph[:], wsb[i][:, c4, of * 128:(of + 1) * 128],
                                             ytn[:, c4, :],
                                             start=(c4 == 0), stop=(c4 == 3))
                        nc.scalar.activation(htile[:, of, :], ph[:],
                                             TANH, bias=bias[:, i * 4 + of: i * 4 + of + 1],
                                             scale=1.0)
                    for om in range(2):
                        pacc = pp.tile([128, 512], f32, name=f"pa{i}_{nt}_{om}", tag="acc", bufs=3)
                        for hf in range(4):
                            nc.tensor.matmul(pacc[:], wl[:, i * 4 + hf, om * 128:(om + 1) * 128],
                                             htile[:, hf, :],
                                             start=(hf == 0), stop=(hf == 3))
                        dst = oacc[:, om, nt * 512:(nt + 1) * 512]
                        if i == 0:
                            nc.vector.tensor_copy(dst, pacc[:])
                        else:
                            nc.vector.tensor_add(dst, dst, pacc[:])

                    if i == G - 1:
                        # output epilogue for this node tile: +b_lin, transpose
                        # back to node-major, DMA out
                        po = pp.tile([128, 4, OUT], bf16, name=f"po{nt}", tag="otp", bufs=2)
                        otb = []
                        for om in range(2):
                            ob = wp.tile([128, 512], bf16, name=f"ot{nt}_{om}", tag="ot", bufs=2)
                            nc.scalar.activation(ob[:], oacc[:, om, nt * 512:(nt + 1) * 512],
                                                 IDENT, bias=bias[:, 12 + om: 13 + om],
                                                 scale=1.0)
                            otb.append(ob)
                        n8 = 0
                        for om in range(2):
                            for j in range(4):
                                nc.tensor.matmul(po[:, j, om * 128:(om + 1) * 128],
                                                 otb[om][:, j * 128:(j + 1) * 128], ident[:],
                                                 is_transpose=True,
                                                 start=(n8 == 0), stop=(n8 == 7))
                                n8 += 1
                        off = wp.tile([128, 4, OUT], f32, name=f"off{nt}", tag="of", bufs=2)
                        nc.scalar.copy(off[:], po[:])
                        nc.sync.dma_start(
                            out_d.ap().rearrange("(t j p) o -> t p j o", p=128, j=4)[nt],
                            off[:],
                        )

    nc.compile()
    return nc


def kernel(**inputs) -> np.ndarray:
    in_maps, K_all, devrows = _prepare(inputs)
    key = tuple(tuple(k) for k in K_all)
    nc = _CACHE.get(key)
    if nc is None:
        nc = _build(K_all)
        _CACHE.clear()
        _CACHE[key] = nc
    res = run_bass_kernel_spmd(nc, in_maps, core_ids=list(range(NCORES)))
    out = np.concatenate(
        [np.asarray(res.results[c]["out"])[devrows[c]] for c in range(NCORES)], axis=0
    )
    return out.astype(np.float32)
